# revision 10
# baseline (speedup 1.0000x reference)
"""Multi-head self-attention on Trainium2, 8-core SPMD.

Problem: x[2,2048,1024] -> torch-style MHSA (16 heads, head_dim 64) -> [2,2048,1024]

Sharding (data + tensor parallel): 8 cores = 2 batches x 4 head-groups.
Each core handles one batch and 4 heads: it computes Q/K/V projections for
its 256 channels, attention for its 4 heads, and the out-projection with its
256 rows of Wo, producing a partial [S, E] output (bf16). The host sums the
4 head-group partials per batch and adds the output bias.

Kernel design (v7 — DR M=128 merged PV+denominator, balanced engines,
multi-queue DMA, tail-heavy out-projection):
  * Fused pipeline: PE emits K-projection, Q(block0), V st0-11, then runs
    attention; remaining V tiles and later Q blocks interleave between
    attention steps as PE filler. All 32 out-projection tiles run as a
    dense PE tail after the attention loop (their psum drains would
    otherwise push the exp engines over the per-block budget), with
    drains alternating ACT/DVE and out DMAs alternating queues; the tail
    starts with qb0-2 work so the final rcp-bounce round trip is hidden.
  * Scores in bf16, transposed orientation (lhsT=K^T, rhs=Q^T, contraction
    dh=64 on PE rows 0-63/64-127 per head of a pair), one [128,QB] psum
    bank per (key-tile, head); 1/sqrt(dh) folded into Wq/bq on the host.
  * PV contraction in fp8e4m3 DoubleRow with M=128: the stationary weights
    for each (key-tile-pair, head) are [8*V_h(64 cols) | ones(64 cols)], so
    psum rows 0-63 accumulate the numerator and rows 64-127 the softmax
    denominator in the SAME chain — no separate denominator matmuls.
    (M=128 is accepted by walrus and correct on HW; only 65..127 fail.)
  * attn probs stored fp8: ACT exp writes float8e4 directly for 17/32 of
    (key-tile, head) slots; the other 15 are exp'd on DVE via a
    Schraudolph fused multiply-add whose uint8 output bits ARE the
    fp8e4m3 value (i8 = round(s*8/ln2 + 55.62), ~1.9% rms), balancing
    the two psum-capable engines (GPSIMD cannot access PSUM on TRN2).
    The first four slots of each block are ACT so end-of-block DVE
    drains (pv copy, reciprocals) never delay the next block's exps.
  * Softmax normalization deferred: reciprocal of psum row 64 (bf16),
    partition-broadcast via a DRAM bounce, multiplied into the copied-out
    rows on the otherwise-idle GPSIMD engine. Diagonal scaling commutes
    with the out-projection.
  * Out-projection contracts over 128 channels in 2 accumulation steps
    over pair-stacked outTs[128, pair, S]; even head copied by DVE, odd
    head staged by ACT and hopped to partitions 64-127 by an sbuf->sbuf
    DMA. Output partial is bf16 (halves out DMA; host sums in f64).
  * Two DMA queues: SP carries wk-half + even xt tiles + wq + the
    latency-critical rcp bounce/broadcast/hop traffic; the GPSIMD SWDGE
    queue carries the other wk-half + odd xt tiles + remaining weights +
    half the out drain, so the startup feed runs at 2x and bounces never
    sit behind bulk DMAs.
"""

from contextlib import ExitStack

import numpy as np

import concourse.bacc as bacc
import concourse.mybir as mybir
import concourse.tile as tile

P = 128
DH = 64  # head dim
F32 = mybir.dt.float32
BF = mybir.dt.bfloat16
F8 = mybir.dt.float8e4
U8 = mybir.dt.uint8

# full-size problem constants
FULL_B = 2
FULL_S = 2048
FULL_E = 1024
FULL_H = 16
HPC = 4  # heads per core
N_CORES = 8

V_SCALE = 8.0  # vhat fp8 scale, folded into Wv/bv on host; cancels in softmax
SCHRAUD_A = 8.0 / float(np.log(2.0))
SCHRAUD_B = 55.62

# per-block exp slot position (2*kt + h, 0..31) handled by DVE: 15 slots,
# none in the first four so end-of-block DVE drains never stall the next
# block's first exps.
DVE_POS = frozenset({4, 5, 8, 10, 12, 14, 16, 18, 20, 22, 24, 26, 28, 30, 6})


def exp_engine(kt, h):
    return "dve" if (2 * (kt % 16) + h) in DVE_POS else "act"


def build_nc(S=FULL_S, E=FULL_E, hpc=HPC, reps=1, exp_engine=exp_engine):
    """Build the single-core Bass program (same program on all 8 cores)."""
    assert S % P == 0 and E % P == 0 and hpc % 2 == 0
    HD = hpc * DH            # projected channels per core (256)
    NPAIR = hpc // 2         # head pairs (2)
    EK = E // P              # contraction tiles for projections (8)
    SK = S // P              # key tiles for attention (16)
    SKP = SK // 2            # key tile pairs for the DoubleRow PV (8)
    QB = min(512, S)         # attention query block / proj psum width
    NQB = S // QB
    EMW = min(512, E)        # out-projection matmul moving width
    NEB = E // EMW

    nc = bacc.Bacc(trn_type="TRN2", target_bir_lowering=False, debug=False)

    xT = nc.declare_dram_parameter("xT", [E, S], BF, isOutput=False)
    wq = nc.declare_dram_parameter("wq", [E, HD], BF, isOutput=False)
    wk = nc.declare_dram_parameter("wk", [E, HD], BF, isOutput=False)
    wv = nc.declare_dram_parameter("wv", [E, HD], BF, isOutput=False)
    wo = nc.declare_dram_parameter("wo", [HD, E], BF, isOutput=False)
    bq = nc.declare_dram_parameter("bq", [P, NPAIR], F32, isOutput=False)
    bk = nc.declare_dram_parameter("bk", [P, NPAIR], F32, isOutput=False)
    bvb = nc.declare_dram_parameter("bvb", [P, HD], F32, isOutput=False)
    out = nc.declare_dram_parameter("out", [S, E], BF, isOutput=True)
    rcp_dram = nc.dram_tensor("rcp_scratch", [hpc, S], BF)

    Exp = mybir.ActivationFunctionType.Exp
    Add = mybir.AluOpType.add
    Mult = mybir.AluOpType.mult
    DR = mybir.MatmulPerfMode.DoubleRow

    wk_r = wk.rearrange("(kt p) n -> p kt n", p=P)

    with ExitStack() as ctx:
        tc = ctx.enter_context(tile.TileContext(nc))
        for _rep in range(reps):
            rctx = ctx.enter_context(ExitStack())
            const = rctx.enter_context(tc.tile_pool(name="const", bufs=1))
            proj = rctx.enter_context(tc.tile_pool(name="proj", bufs=1))
            xw = rctx.enter_context(tc.tile_pool(name="xw", bufs=1))
            big_ps = rctx.enter_context(
                tc.tile_pool(name="big_ps", bufs=2, space="PSUM"))
            sc_ps = rctx.enter_context(
                tc.tile_pool(name="sc_ps", bufs=4, space="PSUM"))
            pv_ps = rctx.enter_context(
                tc.tile_pool(name="pv_ps", bufs=1, space="PSUM"))
            at_pool = rctx.enter_context(tc.tile_pool(name="at", bufs=4))
            stg_pool = rctx.enter_context(tc.tile_pool(name="stg", bufs=3))
            rb_pool = rctx.enter_context(tc.tile_pool(name="rb", bufs=2))
            rcp_pool = rctx.enter_context(tc.tile_pool(name="rcp", bufs=4))
            ob_pool = rctx.enter_context(tc.tile_pool(name="ob", bufs=6))

            bq_sb = const.tile([P, NPAIR], F32)
            bk_sb = const.tile([P, NPAIR], F32)
            bv_sb = const.tile([P, HD], F32)
            wo_sb = const.tile([P, NPAIR, E], BF)

            # persistent activation tensors
            qt_sb = proj.tile([P, NPAIR, S], BF)   # Q^T (head pair pr on
            kt_sb = proj.tile([P, NPAIR, S], BF)   # partitions 64pr..), K^T
            # vI: DR M=128 stationary weights. [tp, plane(i), head*128]
            # where each head's 128 columns are [8*V_h(64) | ones(64)].
            vI = proj.tile([P, SKP, 2, hpc * P], F8)
            outTs = proj.tile([P, NPAIR, S], BF)   # pair-stacked attn out^T

            xt = xw.tile([P, EK, S], BF)
            wq_sb = xw.tile([P, EK, HD], BF)
            wk_sb = xw.tile([P, EK, HD], BF)
            wv_sb = xw.tile([P, EK, HD], BF)

            # DMA order = need order; two queues (SP + gpsimd SWDGE) so the
            # startup xt feed runs at 2x and the K path is never starved.
            xT_t = xT.rearrange("(kt p) s -> kt p s", p=P)
            nc.sync.dma_start(out=wk_sb[:, 0:4, :], in_=wk_r[:, 0:4, :])
            nc.gpsimd.dma_start(out=wk_sb[:, 4:8, :], in_=wk_r[:, 4:8, :])
            nc.sync.dma_start(out=xt[:, 0, :], in_=xT_t[0])
            nc.gpsimd.dma_start(out=xt[:, 1, :], in_=xT_t[1])
            nc.sync.dma_start(out=xt[:, 2, :], in_=xT_t[2])
            nc.gpsimd.dma_start(out=xt[:, 3, :], in_=xT_t[3])
            nc.sync.dma_start(out=xt[:, 4, :], in_=xT_t[4])
            nc.gpsimd.dma_start(out=xt[:, 5, :], in_=xT_t[5])
            nc.sync.dma_start(out=xt[:, 6, :], in_=xT_t[6])
            nc.gpsimd.dma_start(out=xt[:, 7, :], in_=xT_t[7])
            nc.gpsimd.dma_start(out=bk_sb[:], in_=bk[:, :])
            nc.gpsimd.dma_start(out=bq_sb[:], in_=bq[:, :])
            nc.sync.dma_start(
                out=wq_sb[:], in_=wq.rearrange("(kt p) n -> p kt n", p=P))
            nc.gpsimd.dma_start(
                out=wv_sb[:], in_=wv.rearrange("(kt p) n -> p kt n", p=P))
            nc.gpsimd.dma_start(out=bv_sb[:], in_=bvb[:, :])
            nc.gpsimd.dma_start(
                out=wo_sb[:], in_=wo.rearrange("(pr p) e -> p pr e", p=P))

            # ones stripes of vI (columns 64-127 of each head block):
            # the denominator weights for the merged PV+den DR chain.
            vI_h = vI.rearrange("p t i (h c) -> p t i h c", c=P)
            nc.vector.memset(vI_h[:, :, :, :, DH:P], 1.0)

            # ---------- building blocks ----------
            ndrain = [0]  # alternates op-psum drains between ACT and DVE

            def qk_proj_tile(w_sb, b_sb, dst, pr, nb, pool=None):
                """dst[:, pr, nb*QB:(nb+1)*QB] = W_pair^T x xT + bias."""
                ssl = slice(nb * QB, (nb + 1) * QB)
                pool = pool or big_ps
                ps = pool.tile([P, QB], F32, tag="sc" if pool is sc_ps else "big")
                for kt in range(EK):
                    nc.tensor.matmul(
                        ps[:],
                        lhsT=w_sb[:, kt, pr * P:(pr + 1) * P],
                        rhs=xt[:, kt, ssl],
                        start=(kt == 0),
                        stop=(kt == EK - 1),
                    )
                nc.vector.tensor_scalar(
                    out=dst[:, pr, ssl], in0=ps[:],
                    scalar1=b_sb[:, pr:pr + 1], scalar2=None, op0=Add)

            def v_proj_tile(st, pool=None):
                """vI[:, st//2, st%2, h*128:h*128+64] = 8*(xT_st^T Wv + bv)_h."""
                pool = pool or big_ps
                ps = pool.tile([P, QB], F32, tag="sc" if pool is sc_ps else "big")
                for kt in range(EK):
                    nc.tensor.matmul(
                        ps[:, 0:HD],
                        lhsT=xt[:, kt, st * P:(st + 1) * P],
                        rhs=wv_sb[:, kt, :],
                        start=(kt == 0),
                        stop=(kt == EK - 1),
                    )
                dst = vI_h[:, st // 2, st % 2, :, 0:DH]
                nc.vector.tensor_add(
                    out=dst,
                    in0=ps[:, 0:HD].rearrange("p (h c) -> p h c", c=DH),
                    in1=bv_sb.rearrange("p (h c) -> p h c", c=DH),
                )

            def op_tile(qb, m, nb):
                """One out-projection psum tile: out[msl, esl], drained to
                bf16 alternately by ACT/DVE, out DMA alternating queues."""
                msl = slice(qb * QB + m * P, qb * QB + (m + 1) * P)
                esl = slice(nb * EMW, (nb + 1) * EMW)
                ps = big_ps.tile([P, EMW], F32, tag="big")
                for pr in range(NPAIR):
                    nc.tensor.matmul(
                        ps[:],
                        lhsT=outTs[:, pr, msl],
                        rhs=wo_sb[:, pr, esl],
                        start=(pr == 0),
                        stop=(pr == NPAIR - 1),
                    )
                ob = ob_pool.tile([P, EMW], BF, tag="ob")
                n = ndrain[0] = ndrain[0] + 1
                if n % 2 == 0:
                    nc.scalar.copy(out=ob[:], in_=ps[:])
                else:
                    nc.vector.tensor_copy(out=ob[:], in_=ps[:])
                eng = nc.gpsimd if n % 2 == 0 else nc.sync
                eng.dma_start(out=out[msl, esl], in_=ob[:])

            # ---------- pre-attention: K, Q0, V st0-11 ----------
            # K-projection in two kt-outer passes of 4 concurrent psum
            # groups each, so the PE consumes every arriving xt tile at
            # once instead of serializing whole psum groups behind the
            # DMA feed of xt
            for half in range(2):
                ktiles = []
                for pr in range(NPAIR):
                    for nb in (2 * half, 2 * half + 1):
                        kp = sc_ps.tile([P, QB], F32, tag="sc")
                        ktiles.append((pr, nb, kp))
                for kt in range(EK):
                    for pr, nb, kp in ktiles:
                        nc.tensor.matmul(
                            kp[:],
                            lhsT=wk_sb[:, kt, pr * P:(pr + 1) * P],
                            rhs=xt[:, kt, nb * QB:(nb + 1) * QB],
                            start=(kt == 0),
                            stop=(kt == EK - 1),
                        )
                for pr, nb, kp in ktiles:
                    nc.vector.tensor_scalar(
                        out=kt_sb[:, pr, nb * QB:(nb + 1) * QB], in0=kp[:],
                        scalar1=bk_sb[:, pr:pr + 1], scalar2=None, op0=Add)
            for pr in range(NPAIR):
                qk_proj_tile(wq_sb, bq_sb, qt_sb, pr, 0, pool=sc_ps)
            for st in range(12):
                v_proj_tile(st, pool=sc_ps)

            # ---------- filler schedule: PE work interleaved into attention
            # (each unit ~0.9-1.7us); consumed at fixed slots inside a block.
            fillers = {
                (0, 0): [lambda: v_proj_tile(12), lambda: v_proj_tile(13),
                         lambda: v_proj_tile(14), lambda: v_proj_tile(15)],
                (0, 1): [lambda: qk_proj_tile(wq_sb, bq_sb, qt_sb, 0, 1),
                         lambda: qk_proj_tile(wq_sb, bq_sb, qt_sb, 1, 1)],
                (1, 0): [lambda: qk_proj_tile(wq_sb, bq_sb, qt_sb, 0, 2),
                         lambda: qk_proj_tile(wq_sb, bq_sb, qt_sb, 1, 2)],
                (2, 0): [lambda: qk_proj_tile(wq_sb, bq_sb, qt_sb, 0, 3),
                         lambda: qk_proj_tile(wq_sb, bq_sb, qt_sb, 1, 3)],
            }
            SLOT_TPS = (1, 3, 5, 7)

            # ---------- attention ----------
            for qb in range(NQB):
                qsl = slice(qb * QB, (qb + 1) * QB)
                for pr in range(NPAIR):
                    h0, h1 = 2 * pr, 2 * pr + 1
                    units = list(fillers.get((qb, pr), ()))
                    pv2 = pv_ps.tile([P, 2, QB], F32, tag="pv")
                    pv0 = pv2[:, 0, :]
                    pv1 = pv2[:, 1, :]
                    for tp in range(SKP):
                        # at4 [P, kt-plane, head, QB] holds both key tiles of
                        # the pair for both heads; the PV DoubleRow matmul
                        # contracts the two kt planes (strided dim1 AP)
                        at4 = at_pool.tile([P, 2, 2, QB], F8, tag="at")
                        for i, kt in enumerate((2 * tp, 2 * tp + 1)):
                            ksl = slice(kt * P, (kt + 1) * P)
                            for h in range(2):
                                hsl = slice(h * DH, (h + 1) * DH)
                                sc = sc_ps.tile([P, QB], F32, tag="sc")
                                nc.tensor.matmul(
                                    sc[:],
                                    lhsT=kt_sb[hsl, pr, ksl],
                                    rhs=qt_sb[hsl, pr, qsl],
                                    start=True, stop=True,
                                    tile_position=(h * DH, 0),
                                )
                                if exp_engine(kt, h) == "act":
                                    nc.scalar.activation(
                                        out=at4[:, i, h, :], in_=sc[:],
                                        func=Exp)
                                else:
                                    # Schraudolph: uint8 bits = fp8e4m3
                                    nc.vector.tensor_scalar(
                                        out=at4[:, i, h, :].bitcast(U8),
                                        in0=sc[:],
                                        scalar1=SCHRAUD_A, scalar2=SCHRAUD_B,
                                        op0=Mult, op1=Add,
                                    )
                        if tp in SLOT_TPS and units:
                            units.pop(0)()
                        # merged PV+den: psum rows 0-63 numerator, 64-127
                        # denominator (ones weight columns)
                        for h, pv in ((0, pv0), (1, pv1)):
                            hb = (2 * pr + h) * P
                            nc.tensor.matmul(
                                pv[:, :],
                                lhsT=vI[:, tp, :, hb:hb + P],
                                rhs=at4[:, :, h, :],
                                start=(tp == 0),
                                stop=(tp == SKP - 1),
                                perf_mode=DR,
                            )
                    while units:
                        units.pop(0)()
                    # copies first: free the pv psum banks for the next pair
                    dst0 = outTs[0:DH, pr, qsl]
                    nc.vector.tensor_copy(out=dst0, in_=pv0[0:DH, :])
                    stg = stg_pool.tile([DH, QB], BF, tag="stg")
                    nc.scalar.copy(out=stg[:], in_=pv1[0:DH, :])
                    # ONE reciprocal + bounce for both heads (row 64 of each
                    # pv bank via the 2-bank AP)
                    rcp = rcp_pool.tile([1, 2, QB], BF, tag="rcp")
                    with nc.allow_low_precision(
                            reason="softmax denom reciprocal in bf16"):
                        nc.vector.reciprocal(
                            out=rcp[:], in_=pv2[DH:DH + 1, :, :])
                    # partition-broadcast needs a DRAM source
                    nc.sync.dma_start(
                        out=rcp_dram[h0:h0 + 2, qsl],
                        in_=rcp[:])
                    for h, dsl in ((h0, dst0), (h1, stg[:, :])):
                        odd = h % 2
                        rb = rb_pool.tile([DH, QB], BF, tag="rb")
                        nc.sync.dma_start(
                            out=rb[:],
                            in_=rcp_dram[h:h + 1, qsl].to_broadcast((DH, QB)),
                        )
                        nc.gpsimd.tensor_mul(out=dsl, in0=dsl, in1=rb[:])
                        if odd:
                            # hop the odd head to partitions 64-127
                            nc.sync.dma_start(
                                out=outTs[DH:P, pr, qsl], in_=dsl)

            # tail: all out-projections, oldest q-blocks first so the final
            # rcp-bounce round trip hides behind ready work.
            for qb in range(NQB):
                for m in range(QB // P):
                    for nb in range(NEB):
                        op_tile(qb, m, nb)

            rctx.close()

    nc.compile()
    return nc


def make_in_maps(x, Wq, bq, Wk, bk, Wv, bv, Wo, hpc=HPC, n_cores=N_CORES):
    """Host-side sharding: per-core input dict list."""
    import ml_dtypes
    bf16 = ml_dtypes.bfloat16
    x = np.asarray(x, dtype=np.float32)
    B = x.shape[0]
    groups = n_cores // B
    HD = hpc * DH
    scale = 1.0 / np.sqrt(np.float32(DH))
    in_maps = []
    for c in range(n_cores):
        b, g = divmod(c, groups)
        hs = slice(g * HD, (g + 1) * HD)
        bq_s = (np.asarray(bq)[hs] * scale).astype(np.float32)
        bk_s = np.asarray(bk)[hs].astype(np.float32)
        bv_s = (np.asarray(bv)[hs] * V_SCALE).astype(np.float32)
        in_maps.append({
            "xT": np.ascontiguousarray(x[b].T).astype(bf16),
            "wq": np.ascontiguousarray(
                np.asarray(Wq)[:, hs] * scale).astype(bf16),
            "wk": np.ascontiguousarray(np.asarray(Wk)[:, hs]).astype(bf16),
            "wv": np.ascontiguousarray(
                np.asarray(Wv)[:, hs] * V_SCALE).astype(bf16),
            "wo": np.ascontiguousarray(
                np.asarray(Wo)[hs, :] / V_SCALE).astype(bf16),
            "bq": np.ascontiguousarray(bq_s.reshape(-1, P).T),
            "bk": np.ascontiguousarray(bk_s.reshape(-1, P).T),
            "bvb": np.ascontiguousarray(
                np.broadcast_to(bv_s, (P, HD))
            ),
        })
    return in_maps


_NC_CACHE = {}


def _get_nc():
    if "nc" not in _NC_CACHE:
        _NC_CACHE["nc"] = build_nc()
    return _NC_CACHE["nc"]


def kernel(x, Wq, bq, Wk, bk, Wv, bv, Wo, bo, _trace=False, _trace_kwargs=None):
    from concourse.bass_utils import run_bass_kernel_spmd

    x = np.asarray(x, dtype=np.float32)
    B, S, E = x.shape
    nc = _get_nc()
    in_maps = make_in_maps(x, Wq, bq, Wk, bk, Wv, bv, Wo)
    res = run_bass_kernel_spmd(
        nc, in_maps, list(range(N_CORES)),
        trace=_trace, **(_trace_kwargs or {}),
    )
    groups = N_CORES // B
    full = np.zeros((B, S, E), dtype=np.float64)
    for c in range(N_CORES):
        full[c // groups] += np.asarray(res.results[c]["out"], dtype=np.float64)
    full += np.asarray(bo, dtype=np.float64)
    out = full.astype(np.float32)
    if _trace:
        return out, res
    return out
